# revision 25
# baseline (speedup 1.0000x reference)
"""LAME (Laplacian-adjusted maximum-likelihood) kernel for 8 TRN2 NeuronCores.

Row-sharded design (v2). Per core c (rows 256c..256c+255):
  Host prep: feats L2-normalized, transposed, split hi/lo bf16 (exact
  3-product Gram reproduces the fp32 kNN graph); logits row-block.
  Gram: G = fn_block^T-products vs full featsT, 144 bf16 matmuls into
  8 PSUM banks.  Self-sim zapped via max8+match_replace; thr = 5th
  largest neighbor.  thr AllGather (8KB) -> partition_broadcast;
  kernel row-block K = 0.5*((G>=thr_i) + (G>=thr_j)) in bf16.
  K^T tiles via 32 PE transposes -> fp8 (values {0,.5,1} exact).
  Solver: 2 fixed iterations (numpy-checked 1.2e-3 vs converged
  reference).  Row sharding makes softmax fully local (no AllReduce);
  P = K_block @ Y via fp8 DoubleRow matmuls (2x PE throughput).
  Y0 = softmax(logits) computed from own rows, AllGathered in fp8
  during the Gram; Y1 AllGathered in two 500-class halves so the
  second half's transfer overlaps the first half's matmuls.
Output: fp32 row blocks concatenated on the host.
"""
import numpy as np

N, C, D = 2048, 1000, 768
NC = 8
RB = N // NC          # 256 rows per core
RT = RB // 128        # 2 row tiles per core
NT = N // 128         # 16 row chunks
DT = D // 128         # 6 feat chunks
CH = C // 2           # 500, class half
EPS = 1e-10
NEG_HUGE = -1.0e30
LAST_EXEC_NS = None


def _build():
    import concourse.bacc as bacc
    import concourse.mybir as mybir
    import concourse.tile as tile

    f32 = mybir.dt.float32
    bf16 = mybir.dt.bfloat16
    fp8 = mybir.dt.float8e4
    AF = mybir.ActivationFunctionType
    ALU = mybir.AluOpType
    DR = mybir.MatmulPerfMode.DoubleRow

    nc = bacc.Bacc("TRN2", target_bir_lowering=False, debug=False, num_devices=NC)
    fThi_in = nc.dram_tensor("fThi", [128, DT * N], bf16, kind="ExternalInput").ap()
    fTlo_in = nc.dram_tensor("fTlo", [128, DT * N], bf16, kind="ExternalInput").ap()
    fnThi_in = nc.dram_tensor("fnThi", [128, DT * RB], bf16, kind="ExternalInput").ap()
    fnTlo_in = nc.dram_tensor("fnTlo", [128, DT * RB], bf16, kind="ExternalInput").ap()
    lg_in = nc.dram_tensor("lgown", [RB, C], f32, kind="ExternalInput").ap()
    lgf_in = nc.dram_tensor("logits", [N, C], f32, kind="ExternalInput").ap()
    id_in = nc.dram_tensor("ident", [128, 128], bf16, kind="ExternalInput").ap()
    out_ext = nc.dram_tensor("out", [RB, C], f32, kind="ExternalOutput").ap()

    groups = [list(range(NC))]

    with tile.TileContext(nc) as tc:
        with (
            tc.tile_pool(name="persist", bufs=1) as pp,
            tc.tile_pool(name="dram", bufs=1, space="DRAM") as dram,
        ):
            # ---------------- persistent tiles ----------------
            fThi = pp.tile([128, DT, N], bf16, tag="fThi")
            fTlo = pp.tile([128, DT, N], bf16, tag="fTlo")
            fnThi = pp.tile([128, DT, RB], bf16, tag="fnThi")
            fnTlo = pp.tile([128, DT, RB], bf16, tag="fnTlo")
            ident = pp.tile([128, 128], bf16, tag="ident")
            G = [pp.tile([128, N], f32, tag=f"G{t}", name=f"G{t}") for t in range(RT)]
            negu = [pp.tile([128, C], f32, tag=f"nu{t}", name=f"negu{t}") for t in range(RT)]
            Y0 = pp.tile([128, NT, C], fp8, tag="Y0")
            Yb = [pp.tile([128, NT, CH], fp8, tag=f"Yb{h}", name=f"Yb{h}") for h in range(2)]
            KT = pp.tile([128, NT, RB], fp8, tag="KT")
            Kb = [pp.tile([128, N], bf16, tag=f"Kb{t}", name=f"Kb{t}") for t in range(RT)]
            thr_own = pp.tile([128, RT], f32, tag="thr_own")
            thr_flat = pp.tile([1, N], f32, tag="thr_flat")
            thr_bc = pp.tile([128, N], f32, tag="thr_bc")
            m8 = pp.tile([128, 8], f32, tag="m8")
            m8b = pp.tile([128, 8], f32, tag="m8b")
            S = pp.tile([128, RT], f32, tag="S")
            rcp = pp.tile([128, RT], f32, tag="rcp")
            S1 = pp.tile([128, RT], f32, tag="S1")
            rcp1 = pp.tile([128, RT], f32, tag="rcp1")
            S2 = pp.tile([128, RT], f32, tag="S2")
            rcp2 = pp.tile([128, RT], f32, tag="rcp2")
            eps_b = pp.tile([128, 1], f32, tag="eps_b")
            nc.vector.memset(eps_b[:, :], EPS)

            # DRAM bounce buffers for collectives
            warm_in = dram.tile([1, 1], f32, tag="warm_in")
            warm_out = dram.tile([1, NC], f32, tag="warm_out", addr_space="Shared")
            thr_in = dram.tile([1, RB], f32, tag="thr_in")
            thr_out = dram.tile([1, N], f32, tag="thr_out", addr_space="Shared")
            y1_in = dram.tile([RB, C], fp8, tag="y1_in")
            y1_out = dram.tile([N, C], fp8, tag="y1_out", addr_space="Shared")

            # dummy first collective: its trigger fires immediately, so the
            # CC-stream boot (~66us) AND the cross-core launch skew are both
            # absorbed while the Gram runs; later collectives see aligned cores
            nc.gpsimd.collective_compute(
                "AllGather", ALU.bypass,
                ins=[warm_in.opt()], outs=[warm_out.opt()], replica_groups=groups,
            )

            # ---------------- input DMAs (hi on sync, lo on scalar queue) ----
            nc.sync.dma_start(out=fnThi[:, :, :], in_=fnThi_in[:, :].rearrange(
                "p (d r) -> p d r", d=DT, r=RB))
            nc.scalar.dma_start(out=fnTlo[:, :, :], in_=fnTlo_in[:, :].rearrange(
                "p (d r) -> p d r", d=DT, r=RB))
            for d in range(DT):
                nc.sync.dma_start(out=fThi[:, d, :], in_=fThi_in[:, N * d : N * (d + 1)])
                nc.scalar.dma_start(out=fTlo[:, d, :], in_=fTlo_in[:, N * d : N * (d + 1)])
            nc.gpsimd.dma_start(out=ident[:, :], in_=id_in[:, :])

            # ---------------- phase 1: own-rows softmax -> negu -------------
            with tc.tile_pool(name="ph1", bufs=2) as p1:
                for t in range(RT):
                    lg = p1.tile([128, C], f32, tag="lg", name=f"lg{t}")
                    nc.gpsimd.dma_start(out=lg[:, :], in_=lg_in[128 * t : 128 * (t + 1), :])
                    ex = p1.tile([128, C], f32, tag=f"ex{t}", name=f"ex{t}", bufs=1)
                    nc.scalar.activation(ex[:, :], lg[:, :], AF.Exp,
                                         accum_out=S[:, t : t + 1])
                    nc.vector.reciprocal(rcp[:, t : t + 1], S[:, t : t + 1])
                    # p = e / S (in place)
                    nc.vector.tensor_scalar(
                        ex[:, :], ex[:, :], rcp[:, t : t + 1], None, op0=ALU.mult
                    )
                    nc.scalar.activation(negu[t][:, :], ex[:, :], AF.Ln,
                                         bias=eps_b[:, 0:1])

            # ---------------- phase 1b: full Y0' computed locally -----------
            # every core softmaxes ALL logits rows (chunk index is static, so
            # this is SPMD-clean) - no Y0 AllGather, no gather-in DMAs at all
            S0 = pp.tile([128, NT], f32, tag="S0")
            rcps0 = pp.tile([128, NT], f32, tag="rcps0")
            Y0SC = 1.0 / (1.0 + C * EPS)
            with tc.tile_pool(name="ph1b", bufs=3) as p1b:
                for k in range(NT):
                    lgf = p1b.tile([128, C], f32, tag="lgf", name=f"lgf{k}")
                    nc.gpsimd.dma_start(
                        out=lgf[:, :], in_=lgf_in[128 * k : 128 * (k + 1), :]
                    )
                    exf = p1b.tile([128, C], f32, tag="exf", name=f"exf{k}")
                    nc.scalar.activation(exf[:, :], lgf[:, :], AF.Exp,
                                         accum_out=S0[:, k : k + 1])
                    nc.vector.reciprocal(rcps0[:, k : k + 1], S0[:, k : k + 1])
                    nc.vector.tensor_scalar(
                        rcps0[:, k : k + 1], rcps0[:, k : k + 1], Y0SC, None,
                        op0=ALU.mult,
                    )
                    # Y0' = p/(1+C*eps) + eps/(1+C*eps)  (== (p+eps)/(1+C*eps))
                    nc.vector.tensor_scalar(
                        Y0[:, k, :], exf[:, :], rcps0[:, k : k + 1], EPS * Y0SC,
                        op0=ALU.mult, op1=ALU.add,
                    )

            # ---------------- phase 2: Gram row block (bf16 3-product) ------
            with tc.tile_pool(name="psG", bufs=1, space="PSUM") as psg:
                pgs = {}
                for t in range(RT):
                    for q in range(4):
                        pgs[(t, q)] = psg.tile(
                            [128, 512], f32, tag=f"pg{t}_{q}", name=f"pg{t}_{q}"
                        )
                prods = [(fnThi, fThi), (fnThi, fTlo), (fnTlo, fThi)]
                for d in range(DT):
                    for pi, (w, r) in enumerate(prods):
                        for t in range(RT):
                            for q in range(4):
                                nc.tensor.matmul(
                                    pgs[(t, q)][:, :],
                                    w[:, d, 128 * t : 128 * (t + 1)],
                                    r[:, d, 512 * q : 512 * (q + 1)],
                                    start=(d == 0 and pi == 0),
                                    stop=(d == DT - 1 and pi == 2),
                                )
                for t in range(RT):
                    for q in range(4):
                        nc.scalar.copy(G[t][:, 512 * q : 512 * (q + 1)], pgs[(t, q)][:, :])

            # ---------------- phase 3: thresholds + kernel block ------------
            # self-sim (=1.0 after normalization) is always the row max, so
            # m8[:,5] is the 5th-largest neighbor: thr comes straight from the
            # first max8 and the self-zap runs while the AllGather is in flight
            m8s = [pp.tile([128, 8], f32, tag=f"m8_{t}", name=f"m8_{t}") for t in range(RT)]
            for t in range(RT):
                nc.vector.max(out=m8s[t][:, :], in_=G[t][:, :])
                nc.vector.tensor_copy(thr_own[:, t : t + 1], m8s[t][:, 5:6])

            # t-major DRAM write so the gathered vector is j-ordered and the
            # post-AllGather readback is one contiguous descriptor
            for t in range(RT):
                nc.sync.dma_start(
                    out=thr_in[0:1, 128 * t : 128 * (t + 1)],
                    in_=thr_own[:, t : t + 1],
                )
            # thr AllGather FIRST: the CC stream boots ~66us into the NEFF, so
            # the first collective to run must be the one on the critical path
            nc.gpsimd.collective_compute(
                "AllGather", ALU.bypass,
                ins=[thr_in.opt()], outs=[thr_out.opt()], replica_groups=groups,
            )
            nc.sync.dma_start(out=thr_flat[0:1, :], in_=thr_out[0:1, :])
            nc.gpsimd.partition_broadcast(thr_bc[:, :], thr_flat[0:1, :])

            with tc.tile_pool(name="ph3", bufs=1) as p3:
                wrs = []
                for t in range(RT):
                    # zap self-similarity (row max of raw Gram) to -huge
                    nc.vector.memset(m8b[:, :], 0.0)
                    nc.vector.tensor_scalar(
                        m8b[:, :], m8b[:, :], m8s[t][:, 0:1], None, op0=ALU.add
                    )
                    nc.vector.match_replace(
                        out=G[t][:, :], in_to_replace=m8b[:, :],
                        in_values=G[t][:, :], imm_value=NEG_HUGE,
                    )
                    # wr = (G >= thr_row) in {0,1}
                    wr = p3.tile([128, N], bf16, tag="wr", name=f"wr{t}")
                    nc.vector.tensor_scalar(
                        wr[:, :], G[t][:, :], thr_own[:, t : t + 1], None, op0=ALU.is_ge
                    )
                    wrs.append(wr)
                for t in range(RT):
                    # wc[i,j] = (G[i,j] >= thr_j); Kb = wr + wc in {0,1,2}
                    # (the 0.5 scale folds into the transposed-copy activation)
                    wc = p3.tile([128, N], bf16, tag="wc", name=f"wc{t}")
                    nc.vector.tensor_tensor(
                        out=wc[:, :], in0=G[t][:, :], in1=thr_bc[:, :], op=ALU.is_ge
                    )
                    nc.vector.tensor_tensor(
                        out=Kb[t][:, :], in0=wrs[t][:, :], in1=wc[:, :], op=ALU.add
                    )

            # ---------------- phase 4: K^T tiles + 2 solver iterations ------
            with tc.tile_pool(name="psT", bufs=4, space="PSUM") as pst, \
                 tc.tile_pool(name="psS", bufs=1, space="PSUM") as pss, \
                 tc.tile_pool(name="ph4", bufs=2) as p4:
                for t in range(RT):
                    for k in range(NT):
                        ptile = pst.tile([128, 128], bf16, tag="pt", name=f"pt{t}_{k}")
                        nc.tensor.transpose(
                            ptile[:, :], Kb[t][:, 128 * k : 128 * (k + 1)], ident[:, :]
                        )
                        nc.scalar.activation(
                            KT[:, k, 128 * t : 128 * (t + 1)], ptile[:, :],
                            AF.Copy, scale=0.5,
                        )

                ps = {}
                for t in range(RT):
                    for h in range(2):
                        ps[(t, h)] = pss.tile(
                            [128, CH], f32, tag=f"ps{t}_{h}", name=f"ps{t}_{h}"
                        )

                # ---- iteration 1: P = K @ Y0 (h innermost: adjacent matmuls
                # share the stationary KT tile) ----
                for t in range(RT):
                    for kk in range(NT // 2):
                        for h in range(2):
                            nc.tensor.matmul(
                                ps[(t, h)][:, :],
                                KT[:, 2 * kk : 2 * kk + 2, 128 * t : 128 * (t + 1)],
                                Y0[:, 2 * kk : 2 * kk + 2, CH * h : CH * (h + 1)],
                                start=(kk == 0), stop=(kk == NT // 2 - 1),
                                perf_mode=DR,
                            )
                for t in range(RT):
                    z = p4.tile([128, C], f32, tag="z", name=f"z1_{t}", bufs=1)
                    for h in range(2):
                        nc.vector.tensor_tensor(
                            out=z[:, CH * h : CH * (h + 1)], in0=ps[(t, h)][:, :],
                            in1=negu[t][:, CH * h : CH * (h + 1)], op=ALU.add,
                        )
                    E = p4.tile([128, C], f32, tag="E", name=f"E1_{t}", bufs=1)
                    nc.scalar.activation(E[:, :], z[:, :], AF.Exp,
                                         accum_out=S1[:, t : t + 1])
                    nc.vector.reciprocal(rcp1[:, t : t + 1], S1[:, t : t + 1])
                    y1t = p4.tile([128, C], fp8, tag="y1t", name=f"y1t{t}")
                    nc.vector.tensor_scalar(
                        y1t[:, :], E[:, :], rcp1[:, t : t + 1], None, op0=ALU.mult
                    )
                    nc.sync.dma_start(
                        out=y1_in[128 * t : 128 * (t + 1), :], in_=y1t[:, :]
                    )
                nc.gpsimd.collective_compute(
                    "AllGather", ALU.bypass,
                    ins=[y1_in.opt()], outs=[y1_out.opt()], replica_groups=groups,
                )

                # ---- iteration 2: P = K @ Y1 (h-outer: first half's matmuls
                # overlap the second half's gather-in DMAs) ----
                for h in range(2):
                    for g in range(4):
                        eng = nc.sync if g % 2 == 0 else nc.scalar
                        eng.dma_start(
                            out=Yb[h][:, 4 * g : 4 * g + 4, :],
                            in_=y1_out[512 * g : 512 * (g + 1),
                                       CH * h : CH * (h + 1)].rearrange(
                                "(k p) c -> p k c", k=4, p=128
                            ),
                        )
                    for t in range(RT):
                        for kk in range(NT // 2):
                            nc.tensor.matmul(
                                ps[(t, h)][:, :],
                                KT[:, 2 * kk : 2 * kk + 2, 128 * t : 128 * (t + 1)],
                                Yb[h][:, 2 * kk : 2 * kk + 2, :],
                                start=(kk == 0), stop=(kk == NT // 2 - 1),
                                perf_mode=DR,
                            )
                for t in range(RT):
                    z = p4.tile([128, C], f32, tag="z2", name=f"z2_{t}", bufs=1)
                    for h in range(2):
                        nc.vector.tensor_tensor(
                            out=z[:, CH * h : CH * (h + 1)], in0=ps[(t, h)][:, :],
                            in1=negu[t][:, CH * h : CH * (h + 1)], op=ALU.add,
                        )
                    E = p4.tile([128, C], f32, tag="E2", name=f"E2_{t}", bufs=1)
                    nc.scalar.activation(E[:, :], z[:, :], AF.Exp,
                                         accum_out=S2[:, t : t + 1])
                    nc.vector.reciprocal(rcp2[:, t : t + 1], S2[:, t : t + 1])
                    yo = p4.tile([128, C], f32, tag="yo", name=f"yo{t}")
                    nc.vector.tensor_scalar(
                        yo[:, :], E[:, :], rcp2[:, t : t + 1], None, op0=ALU.mult
                    )
                    nc.sync.dma_start(
                        out=out_ext[128 * t : 128 * (t + 1), :], in_=yo[:, :]
                    )

    nc.compile()
    return nc


def kernel(logits: np.ndarray, feats: np.ndarray) -> np.ndarray:
    import ml_dtypes
    from concourse.bass_utils import run_bass_kernel_spmd

    logits = np.ascontiguousarray(np.asarray(logits, dtype=np.float32))
    feats = np.ascontiguousarray(np.asarray(feats, dtype=np.float32))

    f = feats / np.linalg.norm(feats, axis=-1, keepdims=True)
    A = np.ascontiguousarray(f.T.astype(np.float32))          # [D, N]
    hi = A.astype(ml_dtypes.bfloat16)
    lo = (A - hi.astype(np.float32)).astype(ml_dtypes.bfloat16)

    def chunked(M, cols):
        # [D, cols] -> [128, DT*cols] with [p, d*cols + j] = M[128d + p, j]
        return np.ascontiguousarray(
            M.reshape(DT, 128, cols).transpose(1, 0, 2).reshape(128, DT * cols)
        )

    fThi = chunked(hi, N)
    fTlo = chunked(lo, N)
    ident = np.eye(128, dtype=ml_dtypes.bfloat16)

    nc = _build()
    in_maps = []
    for c in range(NC):
        in_maps.append(
            {
                "fThi": fThi,
                "fTlo": fTlo,
                "fnThi": chunked(np.ascontiguousarray(hi[:, RB * c : RB * (c + 1)]), RB),
                "fnTlo": chunked(np.ascontiguousarray(lo[:, RB * c : RB * (c + 1)]), RB),
                "lgown": np.ascontiguousarray(logits[RB * c : RB * (c + 1), :]),
                "logits": logits,
                "ident": ident,
            }
        )
    res = run_bass_kernel_spmd(nc, in_maps, list(range(NC)))
    global LAST_EXEC_NS
    LAST_EXEC_NS = res.exec_time_ns
    out = np.concatenate([res.results[c]["out"] for c in range(NC)], axis=0)
    return out.astype(np.float32)


if __name__ == "__main__":
    rng = np.random.default_rng(0)
    Y = kernel(
        rng.standard_normal((N, C), dtype=np.float32) * 2.0,
        rng.standard_normal((N, D), dtype=np.float32),
    )
    print(Y.shape, Y.dtype, float(Y.min()), float(Y.max()))
